# revision 1
# baseline (speedup 1.0000x reference)
"""Trainium2 Bass kernel for nn_STTM_Single (cross-attention + conv1x1 tail).

Reference computation (per batch b, row h; positions w/x along width W=320):
    q = wq @ left[:, w]   k = wk @ right[:, x]   v = wv @ right[:, x]
    dots[w, x] = (q[:, w] . k[:, x]) * 64**-0.5
    attn = softmax_x(dots)
    out[c, w] = sum_x attn[w, x] v[c, x]
    y = BN(w1 @ concat(left, out)) -> LeakyReLU(0.2) -> w2 @ y

Algebraic folds done on the host (fp64) so the device kernel is lean:
    dots = left^T (0.125 * wq^T wk) right        -> one fused matrix G
    w1 @ concat(left, out) = w1a@left + (w1b wv) @ right-weighted-attn
        with U = s*w1b @ wv, so the 512-wide v/attn-out never materializes
    BN (eval mode) folded into w1 row scale s and a bias vector.

Per-core device pipeline (24 (b,h) rows per core, 8 cores over H=96).
All matmul operands are fp16 (full PE rate, fp32 PSUM accumulation);
softmax statistics and the LeakyReLU epilogue run in fp32.
    KG   = G @ Xr                  [512, 320]   (PE)
    VUT  = Xr^T @ U^T              [320, 256]   (PE, x on partitions)
    dots = Xl^T-slices @ KG        [320, 320]   (PE)
    softmax along free axis (no running max needed: |dots| <= ~20)
    AT   = attn^T                  (PE transpose via identity)
    y    = w1a@Xl + VUT^T@AT; LeakyReLU(y + bias)  (PE + DVE)
    z    = w2 @ y                  -> DMA out (fp32)
"""

import numpy as np

import concourse.bass as bass
import concourse.mybir as mybir
import concourse.tile as tile
from concourse import bass_utils
from concourse.masks import make_identity

B, C, H, W = 2, 512, 96, 320
OUT = 256
N_CORES = 8
H_PER = H // N_CORES  # 12
ROWS = B * H_PER      # 24 (b,h) rows per core
SCALE = 64 ** -0.5
BN_EPS = 1e-5

F32 = mybir.dt.float32
F16 = mybir.dt.float16

# width chunking for the 128-partition dim: 320 = 128 + 128 + 64
W_CH = [(0, 128), (128, 128), (256, 64)]


def _cap_waits(nc: bass.Bass, max_waits: int = 1) -> int:
    """Walrus codegen allows only one sync-wait slot on most instruction
    encodings (DMA descriptors, S3D3 matmul, ...). Tile occasionally attaches
    2-3 waits to one instruction; demote the excess onto standalone
    EventSemaphore instructions (engine-sequencer waits, unlimited) placed
    just before the instruction — and before its paired LDWEIGHTS so the
    weight load stays adjacent to its matmul."""
    n_fixed = 0
    for f in nc.m.functions:
        for blk in f.blocks:
            insts = list(blk.instructions)
            out = []
            for inst in insts:
                kind = inst.__class__.__name__
                si = getattr(inst, "sync_info", None)
                if (
                    kind != "InstEventSemaphore"
                    and si
                    and si.on_wait
                    and len(si.on_wait) > max_waits
                ):
                    waits = list(si.on_wait)
                    excess, keep = waits[:-max_waits], waits[-max_waits:]
                    evs = []
                    for k, w in enumerate(excess):
                        ev = mybir.InstEventSemaphore(
                            name=f"{inst.name}-evw{k}", engine=inst.engine
                        )
                        ev.sync_info = mybir.SyncInfo(on_wait=[w], on_update=[])
                        nc.register_instruction(ev)
                        evs.append(ev)
                    si.on_wait = keep
                    # hop back over an adjacent same-engine LDWEIGHTS pair
                    ip = len(out)
                    while (
                        ip > 0
                        and out[ip - 1].__class__.__name__ == "InstLdweights"
                        and out[ip - 1].engine == inst.engine
                    ):
                        ip -= 1
                    out[ip:ip] = evs
                    n_fixed += 1
                out.append(inst)
            if n_fixed:
                blk.instructions = out
    return n_fixed


def build_nc(rows: int = ROWS) -> bass.Bass:
    nc = bass.Bass()
    lf = nc.declare_dram_parameter("lf", [rows, C, W], F16, isOutput=False)
    rf = nc.declare_dram_parameter("rf", [rows, C, W], F16, isOutput=False)
    gT = nc.declare_dram_parameter("gT", [C, C], F16, isOutput=False)
    uT = nc.declare_dram_parameter("uT", [C, OUT], F16, isOutput=False)
    w1aT = nc.declare_dram_parameter("w1aT", [C, OUT], F16, isOutput=False)
    w2T = nc.declare_dram_parameter("w2T", [OUT, OUT], F16, isOutput=False)
    bnb = nc.declare_dram_parameter("bnb", [OUT], F32, isOutput=False)
    out = nc.declare_dram_parameter("out", [rows, OUT, W], F32, isOutput=True)

    Exp = mybir.ActivationFunctionType.Exp
    AX = mybir.AxisListType.X

    with tile.TileContext(nc) as tc:
        with (
            tc.tile_pool(name="wpool", bufs=1) as wp,
            tc.tile_pool(name="io", bufs=6) as io,
            tc.tile_pool(name="work", bufs=4) as wk,
            tc.tile_pool(name="psum", bufs=1, space="PSUM") as pp,
        ):
            # ---- persistent weights (replicated per core) ----
            gt_sb = wp.tile([128, 4, C], F16, name="gt_sb")
            nc.sync.dma_start(out=gt_sb, in_=gT.rearrange("(j p) m -> p j m", p=128))
            ut_sb = wp.tile([128, 4, OUT], F16, name="ut_sb")
            nc.sync.dma_start(out=ut_sb, in_=uT.rearrange("(j p) m -> p j m", p=128))
            w1a_sb = wp.tile([128, 4, OUT], F16, name="w1a_sb")
            nc.sync.dma_start(out=w1a_sb, in_=w1aT.rearrange("(j p) m -> p j m", p=128))
            w2_sb = wp.tile([128, 2, OUT], F16, name="w2_sb")
            nc.sync.dma_start(out=w2_sb, in_=w2T.rearrange("(j p) m -> p j m", p=128))
            bias_sb = wp.tile([128, 2], F32, name="bias_sb")
            nc.sync.dma_start(out=bias_sb, in_=bnb.rearrange("(j p) -> p j", p=128))
            ident = wp.tile([128, 128], F16, name="ident")
            make_identity(nc, ident)

            for r in range(rows):
                # inputs for this (b, h) row: [c(4x128 partitions), width]
                xr_t = io.tile([128, 4, W], F16, tag="xr", name="xr_t")
                rfv = rf[r].rearrange("(j p) w -> p j w", p=128)
                xl_t = io.tile([128, 4, W], F16, tag="xl", name="xl_t")
                lfv = lf[r].rearrange("(j p) w -> p j w", p=128)
                for j in range(4):  # per-chunk 2D DMAs: 1 queue each, <=2 waits
                    nc.sync.dma_start(out=xr_t[:, j, :], in_=rfv[:, j, :])
                    nc.sync.dma_start(out=xl_t[:, j, :], in_=lfv[:, j, :])

                # ---- KG = G @ Xr : [c1, x] ----
                kg_sb = wk.tile([128, 4, W], F16, tag="kg", name="kg_sb")
                for i in range(4):  # c1 chunk
                    pkg = pp.tile([128, W], F32, tag="pkg", bufs=2, name="pkg")
                    for j in range(4):  # c2 chunk (contraction)
                        nc.tensor.matmul(
                            pkg,
                            gt_sb[:, j, 128 * i : 128 * (i + 1)],
                            xr_t[:, j, :],
                            start=(j == 0),
                            stop=(j == 3),
                        )
                    nc.scalar.copy(kg_sb[:, i, :], pkg)

                # ---- VUT[x, u] = Xr^T @ U^T : x on partitions ----
                vut_sb = wk.tile([128, 3, OUT], F16, tag="vut", name="vut_sb")
                for xc, (x0, xs) in enumerate(W_CH):
                    pvu = pp.tile([128, OUT], F32, tag="pvu", bufs=1, name="pvu")
                    for j in range(4):  # c2 chunk (contraction)
                        nc.tensor.matmul(
                            pvu[:xs, :],
                            xr_t[:, j, x0 : x0 + xs],
                            ut_sb[:, j, :],
                            start=(j == 0),
                            stop=(j == 3),
                        )
                    nc.scalar.copy(vut_sb[:xs, xc, :], pvu[:xs, :])

                # ---- dots + softmax -> attn [w, x] (scale pre-folded in G) ----
                # |scaled dots| <= ~20 for this model, so exp() needs no
                # running-max subtraction; exp goes to fp32, the normalized
                # weights to fp16.
                attn_sb = wk.tile([128, 3, W], F16, tag="attn", name="attn_sb")
                e_sb = wk.tile([128, 3, W], F32, tag="e", name="e_sb")
                stats = wk.tile([128, 3, 4], F32, tag="stats", name="stats")
                for wc, (w0, ws) in enumerate(W_CH):
                    pd = pp.tile([128, W], F32, tag="acc", bufs=3, name="pd")
                    for i in range(4):  # c1 chunk (contraction)
                        nc.tensor.matmul(
                            pd[:ws, :],
                            xl_t[:, i, w0 : w0 + ws],
                            kg_sb[:, i, :],
                            start=(i == 0),
                            stop=(i == 3),
                        )
                    ssum = stats[:ws, wc, 1:2]
                    rs = stats[:ws, wc, 2:3]
                    nc.scalar.activation(
                        e_sb[:ws, wc, :],
                        pd[:ws, :],
                        Exp,
                        accum_out=ssum,
                    )
                    nc.vector.reciprocal(rs, ssum)
                    nc.vector.tensor_scalar_mul(
                        attn_sb[:ws, wc, :], e_sb[:ws, wc, :], rs
                    )

                # ---- AT = attn^T : [x, w] via PE transpose ----
                at_sb = wk.tile([128, 3, W], F16, tag="at", name="at_sb")
                for xc, (x0, xs) in enumerate(W_CH):
                    pt = pp.tile([128, W], F16, tag="misc", bufs=2, name="pt")
                    for wc, (w0, ws) in enumerate(W_CH):
                        nc.tensor.transpose(
                            pt[:xs, w0 : w0 + ws],
                            attn_sb[:ws, wc, x0 : x0 + xs],
                            ident[:ws, :ws],
                        )
                    nc.vector.tensor_copy(at_sb[:xs, xc, :], pt[:xs, :])

                # ---- y = w1a @ Xl + VUT^T @ AT ; LeakyReLU(y + bias) ----
                y_sb = wk.tile([128, 2, W], F16, tag="y", name="y_sb")
                u_sb = wk.tile([128, W], F32, tag="lr", name="u_sb")
                for uc in range(2):
                    py = pp.tile([128, W], F32, tag="acc", bufs=3, name="py")
                    for i in range(4):  # c1 contraction (w1a part)
                        nc.tensor.matmul(
                            py,
                            w1a_sb[:, i, 128 * uc : 128 * (uc + 1)],
                            xl_t[:, i, :],
                            start=(i == 0),
                            stop=False,
                        )
                    for xc, (x0, xs) in enumerate(W_CH):  # x contraction (attn part)
                        nc.tensor.matmul(
                            py,
                            vut_sb[:xs, xc, 128 * uc : 128 * (uc + 1)],
                            at_sb[:xs, xc, :],
                            start=False,
                            stop=(xc == 2),
                        )
                    # LeakyReLU(t) = max(t, 0.2*t) with t = py + bias
                    bias_ap = bias_sb[:, uc : uc + 1]
                    nc.vector.tensor_scalar(
                        out=u_sb,
                        in0=py,
                        scalar1=bias_ap,
                        scalar2=0.2,
                        op0=mybir.AluOpType.add,
                        op1=mybir.AluOpType.mult,
                    )
                    nc.vector.scalar_tensor_tensor(
                        out=y_sb[:, uc, :],
                        in0=py,
                        scalar=bias_ap,
                        in1=u_sb,
                        op0=mybir.AluOpType.add,
                        op1=mybir.AluOpType.max,
                    )

                # ---- z = w2 @ y -> DRAM ----
                z_sb = wk.tile([128, 2, W], F32, tag="z", name="z_sb")
                for oc in range(2):
                    pz = pp.tile([128, W], F32, tag="misc", bufs=2, name="pz")
                    for uc in range(2):
                        nc.tensor.matmul(
                            pz,
                            w2_sb[:, uc, 128 * oc : 128 * (oc + 1)],
                            y_sb[:, uc, :],
                            start=(uc == 0),
                            stop=(uc == 1),
                        )
                    nc.vector.tensor_copy(z_sb[:, oc, :], pz)
                outv = out[r].rearrange("(j p) w -> p j w", p=128)
                for oc in range(2):
                    nc.sync.dma_start(out=outv[:, oc, :], in_=z_sb[:, oc, :])
    _cap_waits(nc)
    return nc


def fold_weights(wq, wk, wv, w1, bn_gamma, bn_beta, bn_mean, bn_var, w2):
    """Host-side fp64 weight folding; returns the small device tensors."""
    f8 = np.float64
    s = bn_gamma.astype(f8) / np.sqrt(bn_var.astype(f8) + BN_EPS)
    w1s = w1.astype(f8) * s[:, None]
    w1a = w1s[:, :C]           # applies to left_feat
    w1b = w1s[:, C:]           # applies to the attention output
    U = w1b @ wv.astype(f8)    # [OUT, C]
    gTm = SCALE * (wk.astype(f8).T @ wq.astype(f8))  # [c2, c1]
    bias = bn_beta.astype(f8) - bn_mean.astype(f8) * s
    return {
        "gT": np.ascontiguousarray(gTm, np.float16),
        "uT": np.ascontiguousarray(U.T, np.float16),
        "w1aT": np.ascontiguousarray(w1a.T, np.float16),
        "w2T": np.ascontiguousarray(w2.astype(f8).T, np.float16),
        "bnb": np.ascontiguousarray(bias, np.float32),
    }


def make_in_maps(inputs):
    left = np.asarray(inputs["left_feat"], np.float16)
    right = np.asarray(inputs["right_feat"], np.float16)
    common = fold_weights(
        np.asarray(inputs["wq"]),
        np.asarray(inputs["wk"]),
        np.asarray(inputs["wv"]),
        np.asarray(inputs["w1"]),
        np.asarray(inputs["bn_gamma"]),
        np.asarray(inputs["bn_beta"]),
        np.asarray(inputs["bn_mean"]),
        np.asarray(inputs["bn_var"]),
        np.asarray(inputs["w2"]),
    )
    in_maps = []
    for core in range(N_CORES):
        hs = slice(core * H_PER, (core + 1) * H_PER)
        lf = left[:, :, hs, :].transpose(0, 2, 1, 3).reshape(ROWS, C, W)
        rf = right[:, :, hs, :].transpose(0, 2, 1, 3).reshape(ROWS, C, W)
        in_maps.append(
            {
                "lf": np.ascontiguousarray(lf),
                "rf": np.ascontiguousarray(rf),
                **common,
            }
        )
    return in_maps


def assemble_out(results):
    out = np.empty((B, OUT, H, W), np.float32)
    for core in range(N_CORES):
        o = np.asarray(results[core]["out"]).reshape(B, H_PER, OUT, W)
        out[:, :, core * H_PER : (core + 1) * H_PER, :] = o.transpose(0, 2, 1, 3)
    return out


_NC_CACHE: dict[int, bass.Bass] = {}


def get_nc(rows: int = ROWS) -> bass.Bass:
    if rows not in _NC_CACHE:
        _NC_CACHE[rows] = build_nc(rows)
    return _NC_CACHE[rows]


def run_sharded(inputs, **run_kwargs) -> bass_utils.BassKernelResults:
    """Run the SPMD kernel on all 8 cores; extra kwargs go to the runner
    (e.g. trace=True, trace_cores=[0] for NTFF profiling in test.py)."""
    in_maps = make_in_maps(inputs)
    nc = get_nc()
    return bass_utils.run_bass_kernel_spmd(
        nc, in_maps, core_ids=list(range(N_CORES)), **run_kwargs
    )


def kernel(**inputs) -> np.ndarray:
    return assemble_out(run_sharded(inputs).results)

